# revision 23
# baseline (speedup 1.0000x reference)
"""Trainium2 Bass kernel for nn_Mlp_StaticRoutedLoRAExpert.

Computation (per token chunk with static expert e):
    h = gelu(x @ w1.T + bias1 + SCALE * (x @ a1[e].T) @ b1[e].T)
    y = h @ w2.T + bias2 + SCALE * (h @ a2[e].T) @ b2[e].T

Design:
  * LoRA folded into the dense weights on the host:
        W1_eff[e] = w1 + SCALE * b1[e] @ a1[e]   (same for W2_eff)
    so the device kernel is a plain per-chunk-expert MLP.
  * Data-parallel over batch: 4 batch rows per core on 8 cores.
  * Tokens are host-packed grouped by expert, so each core runs
    expert-contiguous T=512 tiles with a single weight switch, and each
    tile's x load / y store is ONE large contiguous DMA.
  * All matmul operands bf16 (fp32 PSUM accumulate), single fused pass:
    h stays in SBUF - no DRAM round trip for the hidden activations.
"""

import numpy as np
import ml_dtypes

BF16 = ml_dtypes.bfloat16

SCALE = 128.0 / 64.0
B, S, IN, HID, OUT, E, R = 32, 1280, 768, 3072, 768, 2, 64
NCORES = 8
BPC = B // NCORES          # batch rows per core
TPC = BPC * S              # tokens per core
P = 128
KI = IN // P               # 6
KH = HID // P              # 24
KO = OUT // P              # 6
MAX_T = 512                # PSUM bank / fp32 moving-operand limit

_nc_cache: dict = {}


def _segments(chunk_sizes, eids):
    """Packed-order segments (batch_row, seq_start, length, expert):
    chunks sorted by expert id (stable), each expanded over batch rows."""
    order = sorted(range(len(eids)), key=lambda i: (eids[i], i))
    segs = []
    for ci in order:
        s0 = int(sum(chunk_sizes[:ci]))
        for b in range(BPC):
            segs.append((b, s0, int(chunk_sizes[ci]), int(eids[ci])))
    return segs


def _plan_tiles(chunk_sizes, eids):
    """Per-core tiles over the packed token stream: (tok_off, T, expert)."""
    segs = _segments(chunk_sizes, eids)
    tiles = []
    toff = 0
    i = 0
    while i < len(segs):
        e = segs[i][3]
        run = 0
        while i < len(segs) and segs[i][3] == e:
            run += segs[i][2]
            i += 1
        off = 0
        while off < run:
            t = min(MAX_T, run - off)
            tiles.append((toff + off, t, e))
            off += t
        toff += run
    return tuple(tiles)


def _build(tiles, mode="full", internal_io=False, repeat=1,
           psh_bufs=4, split_w1=False, y_engine="sync", y_bf16=True):
    import concourse.bacc as bacc
    import concourse.mybir as mybir
    import concourse.tile as tile

    dt = mybir.dt
    f32 = dt.float32
    bf16 = dt.bfloat16
    AF = mybir.ActivationFunctionType

    nc = bacc.Bacc("TRN2", target_bir_lowering=False, num_devices=NCORES)

    tpc = sum(t for _, t, _ in tiles)
    kin = "Internal" if internal_io else "ExternalInput"
    kout = "Internal" if internal_io else "ExternalOutput"
    if mode == "xread":
        # timing probe: xp stays ExternalInput, everything else Internal;
        # body = L repeats of "DMA all of xp into SBUF".
        kin = "Internal"
        kout = "Internal"

    xp_d = nc.dram_tensor(
        "xp", [P, KI * tpc], bf16,
        kind="ExternalInput" if mode == "xread" else kin,
    )
    w1_d = nc.dram_tensor("w1t", [P, KI, HID], bf16, kind=kin)
    w2_d = nc.dram_tensor("w2t", [P, KH, OUT], bf16, kind=kin)
    a1_d = nc.dram_tensor("a1t", [P, E, KI, R], bf16, kind=kin)
    b1_d = nc.dram_tensor("b1t", [R, E, HID], bf16, kind=kin)
    a2_d = nc.dram_tensor("a2t", [P, E, KH, R], bf16, kind=kin)
    b2_d = nc.dram_tensor("b2t", [R, E, OUT], bf16, kind=kin)
    b1v_d = nc.dram_tensor("bias1", [P, KH], f32, kind=kin)
    b2v_d = nc.dram_tensor("bias2", [P, KO], f32, kind=kin)
    ydt = bf16 if y_bf16 else f32
    yp_d = nc.dram_tensor("yp", [P, KO * tpc], ydt, kind=kout)
    probe_d = None
    if internal_io:
        probe_d = nc.dram_tensor("probe", [1, P], ydt, kind="ExternalOutput")

    do_dma = mode in ("full", "dma")
    do_mm = mode in ("full", "mm")

    # expert runs in tile order: (expert, [tile indices])
    runs = []
    for i, (_, _, e) in enumerate(tiles):
        if runs and runs[-1][0] == e:
            runs[-1][1].append(i)
        else:
            runs.append((e, [i]))

    with tile.TileContext(nc) as tc:
        with (
            tc.tile_pool(name="bias", bufs=1) as bias_pool,
            tc.tile_pool(name="w", bufs=1) as wpool,
            tc.tile_pool(name="xp", bufs=3) as xpool,
            tc.tile_pool(name="hp", bufs=40) as hpool,
            tc.tile_pool(name="yp", bufs=2) as ypool,
            tc.tile_pool(name="psh", bufs=psh_bufs, space="PSUM") as psh,
            tc.tile_pool(name="psy", bufs=2, space="PSUM") as psy,
            tc.tile_pool(name="psu", bufs=2, space="PSUM") as psu,
            tc.tile_pool(name="up", bufs=2) as upool,
        ):
            bias1_s = bias_pool.tile([P, KH], f32, name="bias1s", tag="b1")
            nc.sync.dma_start(bias1_s[:], b1v_d.ap())
            bias2_s = bias_pool.tile([P, KO], f32, name="bias2s", tag="b2")
            nc.sync.dma_start(bias2_s[:], b2v_d.ap())

            w1_s = wpool.tile([P, KI, HID], bf16, name="w1s", tag="w1")
            nc.sync.dma_start(w1_s[:], w1_d.ap())
            w2_s = wpool.tile([P, KH, OUT], bf16, name="w2s", tag="w2")
            nc.sync.dma_start(w2_s[:], w2_d.ap())
            a1_s = wpool.tile([P, E, KI, R], bf16, name="a1s", tag="a1")
            nc.sync.dma_start(a1_s[:], a1_d.ap())
            b1_s = wpool.tile([R, E, HID], bf16, name="b1s", tag="b1l")
            nc.sync.dma_start(b1_s[:], b1_d.ap())
            a2_s = wpool.tile([P, E, KH, R], bf16, name="a2s", tag="a2")
            nc.sync.dma_start(a2_s[:], a2_d.ap())
            b2_s = wpool.tile([R, E, OUT], bf16, name="b2s", tag="b2l")
            nc.sync.dma_start(b2_s[:], b2_d.ap())

            # probe-mode fixed tiles (so every allocated tile has a writer)
            xc_fixed = None
            yc_fixed = None
            xcs_fixed = None
            if mode == "mm":
                xc_fixed = xpool.tile([P, KI * MAX_T], bf16, name="xcf", tag="xc")
                nc.vector.memset(xc_fixed[:], 0.0)
            y_dma = nc.scalar.dma_start if y_engine == "scalar" else nc.sync.dma_start
            if mode == "dma":
                yc_fixed = ypool.tile([P, KO * MAX_T], ydt, name="ycf", tag="yc")
                nc.vector.memset(yc_fixed[:], 0.0)
            if mode in ("dma", "xread"):
                xcs_fixed = [
                    xpool.tile([P, KI * MAX_T], bf16, name=f"xcf{i}", tag="xc")
                    for i in range(3)
                ]

            def body():
                for ti, (toff, T, e) in enumerate(tiles):
                    if True:
                        if mode == "mm":
                            xc = xc_fixed
                        elif mode == "dma":
                            xc = xcs_fixed[ti % 3]
                            nc.sync.dma_start(
                                xc[:, :KI * T],
                                xp_d[:, KI * toff:KI * toff + KI * T],
                            )
                        else:
                            xc = xpool.tile([P, KI * T], bf16, name="xc", tag="xc")
                            if do_dma:
                                nc.sync.dma_start(
                                    xc[:], xp_d[:, KI * toff:KI * toff + KI * T]
                                )
                        u1_s = None
                        if do_mm:
                            u1_ps = psu.tile([R, T], f32, name="u1ps", tag="u")
                            for k in range(KI):
                                nc.tensor.matmul(
                                    u1_ps[:], a1_s[:, e, k, :],
                                    xc[:, k * T:(k + 1) * T],
                                    start=(k == 0), stop=(k == KI - 1),
                                )
                            u1_s = upool.tile([R, T], bf16, name="u1s", tag="u1s")
                            nc.vector.tensor_copy(u1_s[:], u1_ps[:])
                        hcs = []
                        for m in range(KH):
                            hc = None
                            if do_mm:
                                hc = hpool.tile([P, T], bf16, name="hc", tag="hc")
                                h_ps = psh.tile([P, T], f32, name="hps", tag="h")
                                for k in range(KI):
                                    nc.tensor.matmul(
                                        h_ps[:],
                                        w1_s[:, k, m * P:(m + 1) * P],
                                        xc[:, k * T:(k + 1) * T],
                                        start=(k == 0), stop=False,
                                    )
                                nc.tensor.matmul(
                                    h_ps[:], b1_s[:, e, m * P:(m + 1) * P],
                                    u1_s[:], start=False, stop=True,
                                )
                                nc.scalar.activation(
                                    hc[:], h_ps[:], AF.Gelu,
                                    bias=bias1_s[:, m:m + 1],
                                )
                            hcs.append(hc)
                        u2_s = None
                        if do_mm:
                            u2_ps = psu.tile([R, T], f32, name="u2ps", tag="u")
                            for m in range(KH):
                                nc.tensor.matmul(
                                    u2_ps[:], a2_s[:, e, m, :], hcs[m][:],
                                    start=(m == 0), stop=(m == KH - 1),
                                )
                            u2_s = upool.tile([R, T], bf16, name="u2s", tag="u1s")
                            nc.vector.tensor_copy(u2_s[:], u2_ps[:])
                        if mode == "dma":
                            yc = yc_fixed
                        elif mode == "full":
                            yc = ypool.tile([P, KO * T], ydt, name="yc", tag="yc")
                        else:
                            yc = None
                        for o in range(KO):
                            if do_mm:
                                y_ps = psy.tile([P, T], f32, name="yps", tag="y")
                                for m in range(KH):
                                    nc.tensor.matmul(
                                        y_ps[:],
                                        w2_s[:, m, o * P:(o + 1) * P],
                                        hcs[m][:],
                                        start=(m == 0), stop=False,
                                    )
                                nc.tensor.matmul(
                                    y_ps[:], b2_s[:, e, o * P:(o + 1) * P],
                                    u2_s[:], start=False, stop=True,
                                )
                                yv = (
                                    ypool.tile([P, T], ydt, name="yv", tag="yc")
                                    if yc is None else yc[:, o * T:(o + 1) * T]
                                )
                                nc.scalar.activation(
                                    yv, y_ps[:],
                                    AF.Identity, bias=bias2_s[:, o:o + 1],
                                )
                        if do_dma:
                            y_dma(
                                yp_d[:, KO * toff:KO * toff + KO * T],
                                yc[:, :KO * T],
                            )

            def xread_body():
                for ti, (toff, T, _) in enumerate(tiles):
                    xc = xcs_fixed[ti % 3]
                    nc.sync.dma_start(
                        xc[:, :KI * T],
                        xp_d[:, KI * toff:KI * toff + KI * T],
                    )

            if mode == "xread":
                if repeat == 1:
                    xread_body()
                else:
                    with tc.For_i(0, repeat):
                        xread_body()
            elif mode == "empty":
                pass
            elif repeat == 1:
                body()
            else:
                with tc.For_i(0, repeat):
                    body()

        if probe_d is not None:
            nc.sync.dma_start(probe_d.ap(), yp_d[0:1, 0:P])
    nc.compile()
    return nc


def _get_nc(tiles):
    nc = _nc_cache.get(tiles)
    if nc is None:
        nc = _nc_cache[tiles] = _build(tiles)
    return nc


def _pack_weights(w1, bias1, a1, b1, w2, bias2, a2, b2):
    """Lay out dense weights + LoRA tables (SCALE folded into B) for SBUF."""
    out = {
        # [HID, IN] -> w^T [IN, HID] -> [P, KI, HID]
        "w1t": np.ascontiguousarray(
            w1.T.reshape(KI, P, HID).transpose(1, 0, 2)).astype(BF16),
        "w2t": np.ascontiguousarray(
            w2.T.reshape(KH, P, OUT).transpose(1, 0, 2)).astype(BF16),
        # a[e, r, in] -> [P, E, KI, R]; b scaled, [R, E, HID]
        "a1t": np.ascontiguousarray(
            a1.transpose(2, 0, 1).reshape(KI, P, E, R)
            .transpose(1, 2, 0, 3)).astype(BF16),
        "b1t": np.ascontiguousarray(
            (SCALE * b1).transpose(2, 0, 1)).astype(BF16),
        "a2t": np.ascontiguousarray(
            a2.transpose(2, 0, 1).reshape(KH, P, E, R)
            .transpose(1, 2, 0, 3)).astype(BF16),
        "b2t": np.ascontiguousarray(
            (SCALE * b2).transpose(2, 0, 1)).astype(BF16),
        "bias1": np.ascontiguousarray(bias1.reshape(KH, P).T),
        "bias2": np.ascontiguousarray(bias2.reshape(KO, P).T),
    }
    return out


def _run(inputs, trace=False):
    from concourse.bass_utils import run_bass_kernel_spmd

    x = np.asarray(inputs["x"], dtype=np.float32)
    w1 = np.asarray(inputs["w1"], dtype=np.float32)
    bias1 = np.asarray(inputs["bias1"], dtype=np.float32)
    a1 = np.asarray(inputs["a1"], dtype=np.float32)
    b1 = np.asarray(inputs["b1"], dtype=np.float32)
    w2 = np.asarray(inputs["w2"], dtype=np.float32)
    bias2 = np.asarray(inputs["bias2"], dtype=np.float32)
    a2 = np.asarray(inputs["a2"], dtype=np.float32)
    b2 = np.asarray(inputs["b2"], dtype=np.float32)
    chunk_sizes = tuple(int(v) for v in np.asarray(inputs["chunk_sizes"]))
    eids = tuple(int(v) for v in np.asarray(inputs["expert_indices"]))
    assert sum(chunk_sizes) == S

    tiles = _plan_tiles(chunk_sizes, eids)
    segs = _segments(chunk_sizes, eids)
    nc = _get_nc(tiles)

    shared = _pack_weights(w1, bias1, a1, b1, w2, bias2, a2, b2)
    # packed token index within a core: gather x rows in expert-sorted order
    idx = np.concatenate(
        [b * S + s0 + np.arange(sz) for (b, s0, sz, _) in segs]
    )

    in_maps = []
    for c in range(NCORES):
        xc_tok = x[c * BPC:(c + 1) * BPC].reshape(TPC, IN)[idx].astype(BF16)
        xT = np.ascontiguousarray(xc_tok.T)            # [IN, TPC]
        blocks = [
            xT[:, toff:toff + T].reshape(KI, P, T)
            .transpose(1, 0, 2).reshape(P, KI * T)
            for (toff, T, _) in tiles
        ]
        m = dict(shared)
        m["xp"] = np.ascontiguousarray(np.concatenate(blocks, axis=1))
        in_maps.append(m)

    res = run_bass_kernel_spmd(
        nc, in_maps, core_ids=list(range(NCORES)), trace=trace
    )

    y = np.empty((B, S, OUT), np.float32)
    for c in range(NCORES):
        ypk = np.asarray(res.results[c]["yp"]).astype(np.float32)
        yT = np.empty((OUT, TPC), np.float32)
        for (toff, T, _) in tiles:
            yT[:, toff:toff + T] = (
                ypk[:, KO * toff:KO * toff + KO * T]
                .reshape(P, KO, T).transpose(1, 0, 2).reshape(OUT, T)
            )
        ycore = np.empty((TPC, OUT), np.float32)
        ycore[idx] = yT.T
        y[c * BPC:(c + 1) * BPC] = ycore.reshape(BPC, S, OUT)
    return y, res


def kernel(**inputs) -> np.ndarray:
    y, _ = _run(inputs, trace=False)
    return y


# revision 24
# speedup vs baseline: 1.1509x; 1.1509x over previous
"""Trainium2 Bass kernel for nn_Mlp_StaticRoutedLoRAExpert.

Computation (per token chunk with static expert e):
    h = gelu(x @ w1.T + bias1 + SCALE * (x @ a1[e].T) @ b1[e].T)
    y = h @ w2.T + bias2 + SCALE * (h @ a2[e].T) @ b2[e].T

Design:
  * LoRA folded into the dense weights on the host:
        W1_eff[e] = w1 + SCALE * b1[e] @ a1[e]   (same for W2_eff)
    so the device kernel is a plain per-chunk-expert MLP.
  * Data-parallel over batch: 4 batch rows per core on 8 cores.
  * Tokens are host-packed grouped by expert, so each core runs
    expert-contiguous T=512 tiles with a single weight switch, and each
    tile's x load / y store is ONE large contiguous DMA.
  * All matmul operands bf16 (fp32 PSUM accumulate), single fused pass:
    h stays in SBUF - no DRAM round trip for the hidden activations.
"""

import numpy as np
import ml_dtypes

BF16 = ml_dtypes.bfloat16

SCALE = 128.0 / 64.0
B, S, IN, HID, OUT, E, R = 32, 1280, 768, 3072, 768, 2, 64
NCORES = 8
BPC = B // NCORES          # batch rows per core
TPC = BPC * S              # tokens per core
P = 128
KI = IN // P               # 6
KH = HID // P              # 24
KO = OUT // P              # 6
MAX_T = 512                # PSUM bank / fp32 moving-operand limit

_nc_cache: dict = {}


def _segments(chunk_sizes, eids):
    """Packed-order segments (batch_row, seq_start, length, expert):
    chunks sorted by expert id (stable), each expanded over batch rows."""
    order = sorted(range(len(eids)), key=lambda i: (eids[i], i))
    segs = []
    for ci in order:
        s0 = int(sum(chunk_sizes[:ci]))
        for b in range(BPC):
            segs.append((b, s0, int(chunk_sizes[ci]), int(eids[ci])))
    return segs


def _plan_tiles(chunk_sizes, eids):
    """Per-core tiles over the packed token stream: (tok_off, T, expert)."""
    segs = _segments(chunk_sizes, eids)
    tiles = []
    toff = 0
    i = 0
    while i < len(segs):
        e = segs[i][3]
        run = 0
        while i < len(segs) and segs[i][3] == e:
            run += segs[i][2]
            i += 1
        off = 0
        while off < run:
            t = min(MAX_T, run - off)
            tiles.append((toff + off, t, e))
            off += t
        toff += run
    return tuple(tiles)


def _build(tiles, mode="full", internal_io=False, repeat=1,
           psh_bufs=4, split_w1=False, y_engine="sync", y_bf16=True):
    import concourse.bacc as bacc
    import concourse.mybir as mybir
    import concourse.tile as tile

    dt = mybir.dt
    f32 = dt.float32
    bf16 = dt.bfloat16
    AF = mybir.ActivationFunctionType

    nc = bacc.Bacc("TRN2", target_bir_lowering=False, num_devices=NCORES)

    tpc = sum(t for _, t, _ in tiles)
    kin = "Internal" if internal_io else "ExternalInput"
    kout = "Internal" if internal_io else "ExternalOutput"
    if mode == "xread":
        # timing probe: xp stays ExternalInput, everything else Internal;
        # body = L repeats of "DMA all of xp into SBUF".
        kin = "Internal"
        kout = "Internal"

    xp_d = nc.dram_tensor(
        "xp", [P, KI * tpc], bf16,
        kind="ExternalInput" if mode == "xread" else kin,
    )
    w1_d = [nc.dram_tensor(f"w1e{e}", [P, KI, HID], bf16, kind=kin)
            for e in range(E)]
    w2_d = [nc.dram_tensor(f"w2e{e}", [P, KH, OUT], bf16, kind=kin)
            for e in range(E)]
    b1v_d = nc.dram_tensor("bias1", [P, KH], f32, kind=kin)
    b2v_d = nc.dram_tensor("bias2", [P, KO], f32, kind=kin)
    ydt = bf16 if y_bf16 else f32
    yp_d = nc.dram_tensor("yp", [P, KO * tpc], ydt, kind=kout)
    probe_d = None
    if internal_io:
        probe_d = nc.dram_tensor("probe", [1, P], ydt, kind="ExternalOutput")

    do_dma = mode in ("full", "dma")
    do_mm = mode in ("full", "mm")

    # expert runs in tile order: (expert, [tile indices])
    runs = []
    for i, (_, _, e) in enumerate(tiles):
        if runs and runs[-1][0] == e:
            runs[-1][1].append(i)
        else:
            runs.append((e, [i]))

    with tile.TileContext(nc) as tc:
        with (
            tc.tile_pool(name="bias", bufs=1) as bias_pool,
            tc.tile_pool(name="w", bufs=1) as wpool,
            tc.tile_pool(name="xp", bufs=2) as xpool,
            tc.tile_pool(name="hp", bufs=28) as hpool,
            tc.tile_pool(name="yp", bufs=2) as ypool,
            tc.tile_pool(name="psh", bufs=psh_bufs, space="PSUM") as psh,
            tc.tile_pool(name="psy", bufs=2, space="PSUM") as psy,

        ):
            bias1_s = bias_pool.tile([P, KH], f32, name="bias1s", tag="b1")
            nc.sync.dma_start(bias1_s[:], b1v_d.ap())
            bias2_s = bias_pool.tile([P, KO], f32, name="bias2s", tag="b2")
            nc.sync.dma_start(bias2_s[:], b2v_d.ap())

    # expert order as first used by the tile stream, so the second
            # expert's weights stream in behind the first tiles' compute
            eorder = []
            for _, _, e in tiles:
                if e not in eorder:
                    eorder.append(e)
            for e in range(E):
                if e not in eorder:
                    eorder.append(e)
            w1_map, w2_map = {}, {}
            for e in eorder:
                w1_map[e] = wpool.tile([P, KI, HID], bf16, name=f"w1s{e}", tag=f"w1_{e}")
                nc.sync.dma_start(w1_map[e][:], w1_d[e].ap())
                w2_map[e] = wpool.tile([P, KH, OUT], bf16, name=f"w2s{e}", tag=f"w2_{e}")
                nc.sync.dma_start(w2_map[e][:], w2_d[e].ap())

            # probe-mode fixed tiles (so every allocated tile has a writer)
            xc_fixed = None
            yc_fixed = None
            xcs_fixed = None
            if mode == "mm":
                xc_fixed = xpool.tile([P, KI * MAX_T], bf16, name="xcf", tag="xc")
                nc.vector.memset(xc_fixed[:], 0.0)
            y_dma = nc.scalar.dma_start if y_engine == "scalar" else nc.sync.dma_start
            if mode == "dma":
                yc_fixed = ypool.tile([P, KO * MAX_T], ydt, name="ycf", tag="yc")
                nc.vector.memset(yc_fixed[:], 0.0)
            if mode in ("dma", "xread"):
                xcs_fixed = [
                    xpool.tile([P, KI * MAX_T], bf16, name=f"xcf{i}", tag="xc")
                    for i in range(3)
                ]

            def body():
                for ti, (toff, T, e) in enumerate(tiles):
                    if True:
                        if mode == "mm":
                            xc = xc_fixed
                        elif mode == "dma":
                            xc = xcs_fixed[ti % 3]
                            nc.sync.dma_start(
                                xc[:, :KI * T],
                                xp_d[:, KI * toff:KI * toff + KI * T],
                            )
                        else:
                            xc = xpool.tile([P, KI * T], bf16, name="xc", tag="xc")
                            if do_dma:
                                nc.sync.dma_start(
                                    xc[:], xp_d[:, KI * toff:KI * toff + KI * T]
                                )
                        hcs = []
                        for m in range(KH):
                            hc = None
                            if do_mm:
                                hc = hpool.tile([P, T], bf16, name="hc", tag="hc")
                                h_ps = psh.tile([P, T], f32, name="hps", tag="h")
                                for k in range(KI):
                                    nc.tensor.matmul(
                                        h_ps[:],
                                        w1_map[e][:, k, m * P:(m + 1) * P],
                                        xc[:, k * T:(k + 1) * T],
                                        start=(k == 0), stop=(k == KI - 1),
                                    )
                                nc.scalar.activation(
                                    hc[:], h_ps[:], AF.Gelu,
                                    bias=bias1_s[:, m:m + 1],
                                )
                            hcs.append(hc)
                        if mode == "dma":
                            yc = yc_fixed
                        elif mode == "full":
                            yc = ypool.tile([P, KO * T], ydt, name="yc", tag="yc")
                        else:
                            yc = None
                        for o in range(KO):
                            if do_mm:
                                y_ps = psy.tile([P, T], f32, name="yps", tag="y")
                                for m in range(KH):
                                    nc.tensor.matmul(
                                        y_ps[:],
                                        w2_map[e][:, m, o * P:(o + 1) * P],
                                        hcs[m][:],
                                        start=(m == 0), stop=(m == KH - 1),
                                    )
                                yv = (
                                    ypool.tile([P, T], ydt, name="yv", tag="yc")
                                    if yc is None else yc[:, o * T:(o + 1) * T]
                                )
                                nc.scalar.activation(
                                    yv, y_ps[:],
                                    AF.Identity, bias=bias2_s[:, o:o + 1],
                                )
                        if do_dma:
                            y_dma(
                                yp_d[:, KO * toff:KO * toff + KO * T],
                                yc[:, :KO * T],
                            )

            def xread_body():
                for ti, (toff, T, _) in enumerate(tiles):
                    xc = xcs_fixed[ti % 3]
                    nc.sync.dma_start(
                        xc[:, :KI * T],
                        xp_d[:, KI * toff:KI * toff + KI * T],
                    )

            if mode == "xread":
                if repeat == 1:
                    xread_body()
                else:
                    with tc.For_i(0, repeat):
                        xread_body()
            elif mode == "empty":
                pass
            elif repeat == 1:
                body()
            else:
                with tc.For_i(0, repeat):
                    body()

        if probe_d is not None:
            nc.sync.dma_start(probe_d.ap(), yp_d[0:1, 0:P])
    nc.compile()
    return nc


def _get_nc(tiles):
    nc = _nc_cache.get(tiles)
    if nc is None:
        nc = _nc_cache[tiles] = _build(tiles)
    return nc


def _pack_weights(w1, bias1, a1, b1, w2, bias2, a2, b2):
    """Fold LoRA into dense weights and lay out for SBUF residency."""
    w1e = w1[None, :, :] + SCALE * np.matmul(b1, a1)    # [E, HID, IN]
    w2e = w2[None, :, :] + SCALE * np.matmul(b2, a2)    # [E, OUT, HID]
    out = {}
    for e in range(E):
        out[f"w1e{e}"] = np.ascontiguousarray(
            w1e[e].T.reshape(KI, P, HID).transpose(1, 0, 2)).astype(BF16)
        out[f"w2e{e}"] = np.ascontiguousarray(
            w2e[e].T.reshape(KH, P, OUT).transpose(1, 0, 2)).astype(BF16)
    out["bias1"] = np.ascontiguousarray(bias1.reshape(KH, P).T)
    out["bias2"] = np.ascontiguousarray(bias2.reshape(KO, P).T)
    return out


def _run(inputs, trace=False):
    from concourse.bass_utils import run_bass_kernel_spmd

    x = np.asarray(inputs["x"], dtype=np.float32)
    w1 = np.asarray(inputs["w1"], dtype=np.float32)
    bias1 = np.asarray(inputs["bias1"], dtype=np.float32)
    a1 = np.asarray(inputs["a1"], dtype=np.float32)
    b1 = np.asarray(inputs["b1"], dtype=np.float32)
    w2 = np.asarray(inputs["w2"], dtype=np.float32)
    bias2 = np.asarray(inputs["bias2"], dtype=np.float32)
    a2 = np.asarray(inputs["a2"], dtype=np.float32)
    b2 = np.asarray(inputs["b2"], dtype=np.float32)
    chunk_sizes = tuple(int(v) for v in np.asarray(inputs["chunk_sizes"]))
    eids = tuple(int(v) for v in np.asarray(inputs["expert_indices"]))
    assert sum(chunk_sizes) == S

    tiles = _plan_tiles(chunk_sizes, eids)
    segs = _segments(chunk_sizes, eids)
    nc = _get_nc(tiles)

    shared = _pack_weights(w1, bias1, a1, b1, w2, bias2, a2, b2)
    # packed token index within a core: gather x rows in expert-sorted order
    idx = np.concatenate(
        [b * S + s0 + np.arange(sz) for (b, s0, sz, _) in segs]
    )

    in_maps = []
    for c in range(NCORES):
        xc_tok = x[c * BPC:(c + 1) * BPC].reshape(TPC, IN)[idx].astype(BF16)
        xT = np.ascontiguousarray(xc_tok.T)            # [IN, TPC]
        blocks = [
            xT[:, toff:toff + T].reshape(KI, P, T)
            .transpose(1, 0, 2).reshape(P, KI * T)
            for (toff, T, _) in tiles
        ]
        m = dict(shared)
        m["xp"] = np.ascontiguousarray(np.concatenate(blocks, axis=1))
        in_maps.append(m)

    res = run_bass_kernel_spmd(
        nc, in_maps, core_ids=list(range(NCORES)), trace=trace
    )

    y = np.empty((B, S, OUT), np.float32)
    for c in range(NCORES):
        ypk = np.asarray(res.results[c]["yp"]).astype(np.float32)
        yT = np.empty((OUT, TPC), np.float32)
        for (toff, T, _) in tiles:
            yT[:, toff:toff + T] = (
                ypk[:, KO * toff:KO * toff + KO * T]
                .reshape(P, KO, T).transpose(1, 0, 2).reshape(OUT, T)
            )
        ycore = np.empty((TPC, OUT), np.float32)
        ycore[idx] = yT.T
        y[c * BPC:(c + 1) * BPC] = ycore.reshape(BPC, S, OUT)
    return y, res


def kernel(**inputs) -> np.ndarray:
    y, _ = _run(inputs, trace=False)
    return y


# revision 26
# speedup vs baseline: 1.2326x; 1.0710x over previous
"""Trainium2 Bass kernel for nn_Mlp_StaticRoutedLoRAExpert.

Computation (per token chunk with static expert e):
    h = gelu(x @ w1.T + bias1 + SCALE * (x @ a1[e].T) @ b1[e].T)
    y = h @ w2.T + bias2 + SCALE * (h @ a2[e].T) @ b2[e].T

Design:
  * LoRA folded into the dense weights on the host:
        W1_eff[e] = w1 + SCALE * b1[e] @ a1[e]   (same for W2_eff)
    so the device kernel is a plain per-chunk-expert MLP.
  * Data-parallel over batch: 4 batch rows per core on 8 cores.
  * Tokens are host-packed grouped by expert, so each core runs
    expert-contiguous T=512 tiles with a single weight switch, and each
    tile's x load / y store is ONE large contiguous DMA.
  * All matmul operands bf16 (fp32 PSUM accumulate), single fused pass:
    h stays in SBUF - no DRAM round trip for the hidden activations.
"""

import numpy as np
import ml_dtypes

BF16 = ml_dtypes.bfloat16

SCALE = 128.0 / 64.0
B, S, IN, HID, OUT, E, R = 32, 1280, 768, 3072, 768, 2, 64
NCORES = 8
BPC = B // NCORES          # batch rows per core
TPC = BPC * S              # tokens per core
P = 128
KI = IN // P               # 6
KH = HID // P              # 24
KO = OUT // P              # 6
MAX_T = 512                # PSUM bank / fp32 moving-operand limit

_nc_cache: dict = {}


def _segments(chunk_sizes, eids):
    """Packed-order segments (batch_row, seq_start, length, expert):
    chunks sorted by expert id (stable), each expanded over batch rows."""
    order = sorted(range(len(eids)), key=lambda i: (eids[i], i))
    segs = []
    for ci in order:
        s0 = int(sum(chunk_sizes[:ci]))
        for b in range(BPC):
            segs.append((b, s0, int(chunk_sizes[ci]), int(eids[ci])))
    return segs


def _plan_tiles(chunk_sizes, eids):
    """Per-core tiles over the packed token stream: (tok_off, T, expert)."""
    segs = _segments(chunk_sizes, eids)
    tiles = []
    toff = 0
    i = 0
    while i < len(segs):
        e = segs[i][3]
        run = 0
        while i < len(segs) and segs[i][3] == e:
            run += segs[i][2]
            i += 1
        off = 0
        while off < run:
            t = min(MAX_T, run - off)
            tiles.append((toff + off, t, e))
            off += t
        toff += run
    return tuple(tiles)


def _build(tiles, mode="full", internal_io=False, repeat=1,
           psh_bufs=4, split_w1=False, y_engine="sync", y_bf16=True):
    import concourse.bacc as bacc
    import concourse.mybir as mybir
    import concourse.tile as tile

    dt = mybir.dt
    f32 = dt.float32
    bf16 = dt.bfloat16
    AF = mybir.ActivationFunctionType

    nc = bacc.Bacc("TRN2", target_bir_lowering=False, num_devices=NCORES)

    tpc = sum(t for _, t, _ in tiles)
    kin = "Internal" if internal_io else "ExternalInput"
    kout = "Internal" if internal_io else "ExternalOutput"
    if mode == "xread":
        # timing probe: xp stays ExternalInput, everything else Internal;
        # body = L repeats of "DMA all of xp into SBUF".
        kin = "Internal"
        kout = "Internal"

    xp_d = nc.dram_tensor(
        "xp", [P, KI * tpc], bf16,
        kind="ExternalInput" if mode == "xread" else kin,
    )
    w1_d = [nc.dram_tensor(f"w1e{e}", [P, KI, HID], bf16, kind=kin)
            for e in range(E)]
    w2_d = [nc.dram_tensor(f"w2e{e}", [P, KH, OUT], bf16, kind=kin)
            for e in range(E)]
    b1v_d = nc.dram_tensor("bias1", [P, KH], f32, kind=kin)
    b2v_d = nc.dram_tensor("bias2", [P, KO], f32, kind=kin)
    ydt = bf16 if y_bf16 else f32
    yp_d = nc.dram_tensor("yp", [P, KO * tpc], ydt, kind=kout)
    probe_d = None
    if internal_io:
        probe_d = nc.dram_tensor("probe", [1, P], ydt, kind="ExternalOutput")

    do_dma = mode in ("full", "dma")
    do_mm = mode in ("full", "mm")

    # expert runs in tile order: (expert, [tile indices])
    runs = []
    for i, (_, _, e) in enumerate(tiles):
        if runs and runs[-1][0] == e:
            runs[-1][1].append(i)
        else:
            runs.append((e, [i]))

    with tile.TileContext(nc) as tc:
        with (
            tc.tile_pool(name="bias", bufs=1) as bias_pool,
            tc.tile_pool(name="w", bufs=1) as wpool,
            tc.tile_pool(name="xp", bufs=2) as xpool,
            tc.tile_pool(name="hp", bufs=28) as hpool,
            tc.tile_pool(name="yp", bufs=2) as ypool,
            tc.tile_pool(name="psh", bufs=psh_bufs, space="PSUM") as psh,
            tc.tile_pool(name="psy", bufs=2, space="PSUM") as psy,

        ):
            bias1_s = bias_pool.tile([P, KH], f32, name="bias1s", tag="b1")
            nc.sync.dma_start(bias1_s[:], b1v_d.ap())
            bias2_s = bias_pool.tile([P, KO], f32, name="bias2s", tag="b2")
            nc.sync.dma_start(bias2_s[:], b2v_d.ap())

            # Weight residency for both experts, but only the first tile's
            # fc1 weights load at t=0: the other three 4.7MB tensors are
            # issued from the ACT engine's DMA ring behind tile-0's first
            # activations, so startup HBM bandwidth is dedicated to the
            # critical w1[e0] + x load (trace showed a 44us PE stall when
            # all four streamed concurrently).
            e_first = tiles[0][2] if tiles else 0
            eorder = [e_first] + [e for e in range(E) if e != e_first]
            w1_map, w2_map = {}, {}
            for e in eorder:
                w1_map[e] = wpool.tile(
                    [P, KI, HID], bf16, name=f"w1s{e}", tag=f"w1_{e}")
                w2_map[e] = wpool.tile(
                    [P, KH, OUT], bf16, name=f"w2s{e}", tag=f"w2_{e}")
            nc.sync.dma_start(w1_map[e_first][:], w1_d[e_first].ap())
            deferred = [(w2_map[e_first], w2_d[e_first])] + [
                t for e in eorder if e != e_first
                for t in ((w1_map[e], w1_d[e]), (w2_map[e], w2_d[e]))
            ]
            if mode != "full":
                for ws, wd in deferred:
                    nc.sync.dma_start(ws[:], wd.ap())
                deferred = []

            # probe-mode fixed tiles (so every allocated tile has a writer)
            xc_fixed = None
            yc_fixed = None
            xcs_fixed = None
            if mode == "mm":
                xc_fixed = xpool.tile([P, KI * MAX_T], bf16, name="xcf", tag="xc")
                nc.vector.memset(xc_fixed[:], 0.0)
            y_dma = nc.scalar.dma_start if y_engine == "scalar" else nc.sync.dma_start
            if mode == "dma":
                yc_fixed = ypool.tile([P, KO * MAX_T], ydt, name="ycf", tag="yc")
                nc.vector.memset(yc_fixed[:], 0.0)
            if mode in ("dma", "xread"):
                xcs_fixed = [
                    xpool.tile([P, KI * MAX_T], bf16, name=f"xcf{i}", tag="xc")
                    for i in range(3)
                ]

            def body():
                for ti, (toff, T, e) in enumerate(tiles):
                    if True:
                        if mode == "mm":
                            xc = xc_fixed
                        elif mode == "dma":
                            xc = xcs_fixed[ti % 3]
                            nc.sync.dma_start(
                                xc[:, :KI * T],
                                xp_d[:, KI * toff:KI * toff + KI * T],
                            )
                        else:
                            xc = xpool.tile([P, KI * T], bf16, name="xc", tag="xc")
                            if do_dma:
                                nc.sync.dma_start(
                                    xc[:], xp_d[:, KI * toff:KI * toff + KI * T]
                                )
                        hcs = []
                        for m in range(KH):
                            hc = None
                            if do_mm:
                                hc = hpool.tile([P, T], bf16, name="hc", tag="hc")
                                h_ps = psh.tile([P, T], f32, name="hps", tag="h")
                                for k in range(KI):
                                    nc.tensor.matmul(
                                        h_ps[:],
                                        w1_map[e][:, k, m * P:(m + 1) * P],
                                        xc[:, k * T:(k + 1) * T],
                                        start=(k == 0), stop=(k == KI - 1),
                                    )
                                nc.scalar.activation(
                                    hc[:], h_ps[:], AF.Gelu,
                                    bias=bias1_s[:, m:m + 1],
                                )
                                if ti == 0 and deferred and m % 2 == 0:
                                    ws, wd = deferred.pop(0)
                                    # WAW anchor: scribble one column from
                                    # this activation into the DMA's dest so
                                    # the scheduler cannot hoist the load
                                    # into the startup bandwidth window; the
                                    # DMA then overwrites the whole tile.
                                    nc.vector.tensor_copy(
                                        ws[:, 0, 0:1], hc[:, 0:1])
                                    nc.sync.dma_start(ws[:], wd.ap())
                            hcs.append(hc)
                        if mode == "dma":
                            yc = yc_fixed
                        elif mode == "full":
                            yc = ypool.tile([P, KO * T], ydt, name="yc", tag="yc")
                        else:
                            yc = None
                        for o in range(KO):
                            if do_mm:
                                y_ps = psy.tile([P, T], f32, name="yps", tag="y")
                                for m in range(KH):
                                    nc.tensor.matmul(
                                        y_ps[:],
                                        w2_map[e][:, m, o * P:(o + 1) * P],
                                        hcs[m][:],
                                        start=(m == 0), stop=(m == KH - 1),
                                    )
                                yv = (
                                    ypool.tile([P, T], ydt, name="yv", tag="yc")
                                    if yc is None else yc[:, o * T:(o + 1) * T]
                                )
                                nc.scalar.activation(
                                    yv, y_ps[:],
                                    AF.Identity, bias=bias2_s[:, o:o + 1],
                                )
                        if do_dma:
                            y_dma(
                                yp_d[:, KO * toff:KO * toff + KO * T],
                                yc[:, :KO * T],
                            )

            def xread_body():
                for ti, (toff, T, _) in enumerate(tiles):
                    xc = xcs_fixed[ti % 3]
                    nc.sync.dma_start(
                        xc[:, :KI * T],
                        xp_d[:, KI * toff:KI * toff + KI * T],
                    )

            if mode == "xread":
                if repeat == 1:
                    xread_body()
                else:
                    with tc.For_i(0, repeat):
                        xread_body()
            elif mode == "empty":
                pass
            elif repeat == 1:
                body()
            else:
                with tc.For_i(0, repeat):
                    body()

        if probe_d is not None:
            nc.sync.dma_start(probe_d.ap(), yp_d[0:1, 0:P])
    nc.compile()
    return nc


def _get_nc(tiles):
    nc = _nc_cache.get(tiles)
    if nc is None:
        nc = _nc_cache[tiles] = _build(tiles)
    return nc


def _pack_weights(w1, bias1, a1, b1, w2, bias2, a2, b2):
    """Fold LoRA into dense weights and lay out for SBUF residency."""
    w1e = w1[None, :, :] + SCALE * np.matmul(b1, a1)    # [E, HID, IN]
    w2e = w2[None, :, :] + SCALE * np.matmul(b2, a2)    # [E, OUT, HID]
    out = {}
    for e in range(E):
        out[f"w1e{e}"] = np.ascontiguousarray(
            w1e[e].T.reshape(KI, P, HID).transpose(1, 0, 2)).astype(BF16)
        out[f"w2e{e}"] = np.ascontiguousarray(
            w2e[e].T.reshape(KH, P, OUT).transpose(1, 0, 2)).astype(BF16)
    out["bias1"] = np.ascontiguousarray(bias1.reshape(KH, P).T)
    out["bias2"] = np.ascontiguousarray(bias2.reshape(KO, P).T)
    return out


def _run(inputs, trace=False):
    from concourse.bass_utils import run_bass_kernel_spmd

    x = np.asarray(inputs["x"], dtype=np.float32)
    w1 = np.asarray(inputs["w1"], dtype=np.float32)
    bias1 = np.asarray(inputs["bias1"], dtype=np.float32)
    a1 = np.asarray(inputs["a1"], dtype=np.float32)
    b1 = np.asarray(inputs["b1"], dtype=np.float32)
    w2 = np.asarray(inputs["w2"], dtype=np.float32)
    bias2 = np.asarray(inputs["bias2"], dtype=np.float32)
    a2 = np.asarray(inputs["a2"], dtype=np.float32)
    b2 = np.asarray(inputs["b2"], dtype=np.float32)
    chunk_sizes = tuple(int(v) for v in np.asarray(inputs["chunk_sizes"]))
    eids = tuple(int(v) for v in np.asarray(inputs["expert_indices"]))
    assert sum(chunk_sizes) == S

    tiles = _plan_tiles(chunk_sizes, eids)
    segs = _segments(chunk_sizes, eids)
    nc = _get_nc(tiles)

    shared = _pack_weights(w1, bias1, a1, b1, w2, bias2, a2, b2)
    # packed token index within a core: gather x rows in expert-sorted order
    idx = np.concatenate(
        [b * S + s0 + np.arange(sz) for (b, s0, sz, _) in segs]
    )

    in_maps = []
    for c in range(NCORES):
        xc_tok = x[c * BPC:(c + 1) * BPC].reshape(TPC, IN)[idx].astype(BF16)
        xT = np.ascontiguousarray(xc_tok.T)            # [IN, TPC]
        blocks = [
            xT[:, toff:toff + T].reshape(KI, P, T)
            .transpose(1, 0, 2).reshape(P, KI * T)
            for (toff, T, _) in tiles
        ]
        m = dict(shared)
        m["xp"] = np.ascontiguousarray(np.concatenate(blocks, axis=1))
        in_maps.append(m)

    res = run_bass_kernel_spmd(
        nc, in_maps, core_ids=list(range(NCORES)), trace=trace
    )

    y = np.empty((B, S, OUT), np.float32)
    for c in range(NCORES):
        ypk = np.asarray(res.results[c]["yp"]).astype(np.float32)
        yT = np.empty((OUT, TPC), np.float32)
        for (toff, T, _) in tiles:
            yT[:, toff:toff + T] = (
                ypk[:, KO * toff:KO * toff + KO * T]
                .reshape(P, KO, T).transpose(1, 0, 2).reshape(OUT, T)
            )
        ycore = np.empty((TPC, OUT), np.float32)
        ycore[idx] = yT.T
        y[c * BPC:(c + 1) * BPC] = ycore.reshape(BPC, S, OUT)
    return y, res


def kernel(**inputs) -> np.ndarray:
    y, _ = _run(inputs, trace=False)
    return y
